# revision 17
# baseline (speedup 1.0000x reference)
"""Trainium2 Bass kernel for nn_ItemEmbeddingLayer (fused double-gather + concat).

Vocab-parallel across 8 NeuronCores; core c owns vocab rows
[c*12544, (c+1)*12544). The host routes indices to their owning core,
sorts each core's locals, and ships a fused bf16 table per shard
(emb||genre padded to 512B rows - the dma_gather 256B-multiple minimum
at full descriptor rate). bf16 keeps rel err <= 2^-9 vs the 2e-2 gate.

Primary path (v3, ~218us/core in TimelineSim vs ~1470us for the padded
f32 gather baseline): each index appears ~10.5x, so per 512 sorted
outputs the device gathers only the <=128 DISTINCT rows (one 128-slot
column, ~4x less gather DMA), then expands them with one-hot matmuls:
psum[128 outs, 146] = S^T[slots, outs]^T @ dt[slots, 146]. S^T is built
on the Vector engine by comparing a per-partition iota against the
output->slot map, which a K=1 ones-matmul broadcasts across partitions.
PSUM drains through rolling 3-tiles-per-bank groups (170-f32 pitch so
three 584B matmul outputs fit one 2KB bank) evicted f32->bf16 on the
Activation engine - the binding engine - with each group's 2-tile
remainder alternating onto the Vector engine to balance the two at
~80%. Each group's 4096 rows leave in one 9.3KB-per-partition DMA.

Fallback path (v2, ~315us/core, used only if a 512-output window ever
spans >128 distinct rows - ~12 sigma away for this distribution): plain
dma_gather of every output row, Vector-engine compaction 256->146, same
write scheme.

Host-side: device row order is a static permutation of sorted order, so
unsharding is one fused bf16->f32 cast + scatter per core.
"""
import sys

sys.path.insert(0, "/opt/trn_rl_repo")
import numpy as np
import ml_dtypes

import concourse.bacc as bacc
import concourse.tile as tile
from concourse import mybir
from concourse.bass_utils import run_bass_kernel_spmd

BF16 = np.dtype(ml_dtypes.bfloat16)

P = 128
D, Dg = 128, 18
DF = D + Dg        # 146 real columns per fused row
E = 256            # padded fused row: 146 bf16 -> 256 bf16 (512B, %256)
VSH = 12544        # vocab rows per core shard; 8*12544 >= 100000

# ---- v3 (distinct-gather + matmul expansion) geometry ----
R2G = 4096         # outputs per group
NGRP = 33          # groups per core -> capacity 135168 rows
CAPC = NGRP * R2G
HC = 512           # outputs per half-chunk (one 128-slot distinct column)
NHCG = R2G // HC   # 8 half-chunks per group
NTG = R2G // P     # 32 output tiles per group
NHC = CAPC // HC   # 264 half-chunks per core
W16 = (NHCG * P) // 16  # 64: per-group gather list of 1024 slots, wrapped in 16

# ---- v2 (full gather) geometry ----
G_R2 = 1024        # rows per dma_gather (SWDGE ring-safe)
G_NCH = CAPC // G_R2
G_K = 12           # chunks per grouped idx load
G_NG = G_NCH // G_K
G_W16 = G_R2 // 16
G_NR = G_R2 // P

_nc_cache = {}


def _build_nc_v3():
    nc = bacc.Bacc(None, target_bir_lowering=False, debug=False)
    bf16, i16, f32 = mybir.dt.bfloat16, mybir.dt.int16, mybir.dt.float32
    idx_t = nc.dram_tensor("idx", [NGRP, P, W16], i16, kind="ExternalInput")
    sm_t = nc.dram_tensor("sm", [NGRP, 1, R2G], bf16, kind="ExternalInput")
    tab_t = nc.dram_tensor("tab", [VSH, E], bf16, kind="ExternalInput")
    iota_t = nc.dram_tensor("iota", [P, 1], f32, kind="ExternalInput")
    ones_t = nc.dram_tensor("ones", [1, P], bf16, kind="ExternalInput")
    out_t = nc.dram_tensor("out", [NGRP, P, NTG, DF], bf16, kind="ExternalOutput")
    EP = 170  # padded tile pitch: 3 x 170 f32 = 2040B fits one 2KB PSUM bank
    with tile.TileContext(nc) as tc:
        with (
            tc.tile_pool(name="const", bufs=1) as kpool,
            tc.tile_pool(name="idxp", bufs=3) as ipool,
            tc.tile_pool(name="smp", bufs=3) as spool,
            tc.tile_pool(name="dist", bufs=3) as dpool,
            tc.tile_pool(name="sel", bufs=4) as stpool,
            tc.tile_pool(name="cmp", bufs=2) as cpool,
            tc.tile_pool(name="psb", bufs=3, space="PSUM") as pbpool,
            tc.tile_pool(name="pso", bufs=5, space="PSUM") as popool,
        ):
            iota = kpool.tile([P, 1], f32, name="iota")
            nc.sync.dma_start(out=iota[:], in_=iota_t.ap())
            ones = kpool.tile([1, P], bf16, name="ones")
            nc.sync.dma_start(out=ones[:], in_=ones_t.ap())
            for g in range(NGRP):
                it = ipool.tile([P, W16], i16, name="it")
                nc.sync.dma_start(out=it[:], in_=idx_t.ap()[g])
                sm = spool.tile([1, R2G], bf16, name="sm")
                nc.sync.dma_start(out=sm[:], in_=sm_t.ap()[g])
                dt = dpool.tile([P, NHCG, E], bf16, name="dt")
                # 4 x 256-idx gathers instead of one 1024: the SWDGE ring
                # holds exactly 1024 descriptors, so a full-ring gather
                # serializes gen(g+1) behind transfer(g); quarter gathers
                # let generation overlap the drain (227 -> 221us). NOTE:
                # prefetching these 2 groups ahead wedges the device
                # (NRT_EXEC_UNIT_UNRECOVERABLE) despite simming 5us faster -
                # deep gather queuing overruns the SWDGE ring on real ucode.
                for sg in range(4):
                    nc.gpsimd.dma_gather(
                        out_ap=dt[:, sg * 2:(sg + 1) * 2, :],
                        in_ap=tab_t.ap(),
                        idxs_ap=it[:, sg * (W16 // 4):(sg + 1) * (W16 // 4)],
                        num_idxs=2 * P,
                        num_idxs_reg=2 * P,
                        elem_size=E,
                    )
                ct = cpool.tile([P, NTG, DF], bf16, name="ct")
                # rolling 3-tile PSUM groups: Activation evicts 3 tiles per
                # copy (the binding engine at ~91% busy with 2-tile evicts);
                # the per-group 2-tile remainder goes to the Vector engine,
                # balancing Act/DVE at ~82%/77%.
                cur, fill, base_u = None, 0, 0
                for h in range(NHCG):
                    psb = pbpool.tile([P, HC], f32, name="psb")
                    # broadcast srcmap row across partitions: ones^T @ sm
                    nc.tensor.matmul(psb[:], ones[:], sm[:, h * HC:(h + 1) * HC])
                    # one-hot S^T for all 4 tiles of the half-chunk at once
                    st = stpool.tile([P, HC], bf16, name="st")
                    nc.vector.tensor_scalar(
                        out=st[:],
                        in0=psb[:],
                        scalar1=iota[:],
                        scalar2=None,
                        op0=mybir.AluOpType.is_equal,
                    )
                    for t in range(4):
                        if cur is None:
                            cur = popool.tile([P, 3, EP], f32, name="pso")
                            fill, base_u = 0, h * 4 + t
                        nc.tensor.matmul(
                            cur[:, fill, 0:DF], st[:, t * P:(t + 1) * P], dt[:, h, 0:DF]
                        )
                        fill += 1
                        if fill == 3:
                            nc.scalar.copy(ct[:, base_u:base_u + 3, :], cur[:, :, 0:DF])
                            cur = None
                if cur is not None:
                    # alternate the remainder evict between DVE and Act by
                    # group parity - keeps the two near-equally loaded
                    if g % 2 == 0:
                        nc.vector.tensor_copy(
                            out=ct[:, base_u:base_u + fill, :], in_=cur[:, 0:fill, 0:DF]
                        )
                    else:
                        nc.scalar.copy(ct[:, base_u:base_u + fill, :], cur[:, 0:fill, 0:DF])
                if g == NGRP - 1:
                    # split the final out-DMA so the drain tail starts as soon
                    # as the first quarter of the last group's evicts land
                    for q in range(4):
                        nc.sync.dma_start(
                            out=out_t.ap()[g][:, q * 8:(q + 1) * 8, :],
                            in_=ct[:, q * 8:(q + 1) * 8, :],
                        )
                else:
                    nc.sync.dma_start(out=out_t.ap()[g], in_=ct[:])
    nc.compile()
    return nc


def _build_nc_v2():
    nc = bacc.Bacc(None, target_bir_lowering=False, debug=False)
    bf16, i16 = mybir.dt.bfloat16, mybir.dt.int16
    idx_t = nc.dram_tensor("idx", [G_NG, P, G_K * G_W16], i16, kind="ExternalInput")
    tab_t = nc.dram_tensor("tab", [VSH, E], bf16, kind="ExternalInput")
    out_t = nc.dram_tensor("out", [G_NCH, P, G_NR, DF], bf16, kind="ExternalOutput")
    with tile.TileContext(nc) as tc:
        with (
            tc.tile_pool(name="idxp", bufs=2) as ipool,
            tc.tile_pool(name="rows", bufs=3) as rpool,
            tc.tile_pool(name="cmp", bufs=3) as cpool,
        ):
            for g in range(G_NG):
                it = ipool.tile([P, G_K * G_W16], i16, name="it")
                nc.sync.dma_start(out=it[:], in_=idx_t.ap()[g])
                for k in range(G_K):
                    ch = g * G_K + k
                    rt = rpool.tile([P, G_NR, E], bf16, name="rt")
                    nc.gpsimd.dma_gather(
                        out_ap=rt[:],
                        in_ap=tab_t.ap(),
                        idxs_ap=it[:, k * G_W16:(k + 1) * G_W16],
                        num_idxs=G_R2,
                        num_idxs_reg=G_R2,
                        elem_size=E,
                    )
                    ct = cpool.tile([P, G_NR, DF], bf16, name="ct")
                    nc.vector.tensor_copy(out=ct[:], in_=rt[:, :, 0:DF])
                    nc.scalar.dma_start(out=out_t.ap()[ch], in_=ct[:])
    nc.compile()
    return nc


def _devperm(r2, nr):
    # v3: device flat row j = [g, p, u]; sorted position k = g*r2 + u*128 + p
    # v2: device flat row j = [ch, p, c]; list position k = ch*r2 + c*128 + p
    k = np.arange(CAPC)
    r = k % r2
    return (k - r) + (r % P) * nr + r // P


def _host_prep_v3(loc_sorted_padded):
    """Distinct columns, gather list, srcmap. Raises AssertionError if any
    512-output window spans >128 distinct rows (triggers v2 fallback)."""
    v = loc_sorted_padded.reshape(NHC, HC)
    flags = np.ones_like(v, bool)
    flags[:, 1:] = v[:, 1:] != v[:, :-1]
    srcmap = np.cumsum(flags, axis=1) - 1          # [NHC, HC] 0..nd-1
    assert srcmap[:, -1].max() < P, "distinct overflow"
    cols = np.zeros((NHC, P), np.int16)
    rr, cc = np.nonzero(flags)
    cols[rr, srcmap[rr, cc]] = v[rr, cc]
    glist = cols.reshape(NGRP, NHCG * P)
    idx_w = glist.reshape(NGRP, W16, 16).transpose(0, 2, 1)      # [g, q, f]
    idx_r = np.broadcast_to(idx_w.reshape(NGRP, 1, 16, W16), (NGRP, 8, 16, W16))
    idx_g = idx_r.reshape(NGRP, P, W16).copy()
    sm = srcmap.astype(BF16).reshape(NGRP, 1, R2G)
    return idx_g, sm


def _host_prep_v2(loc_sorted_padded, jperm):
    # place element j at list position k (loc_list[k] = loc[jperm[k]]),
    # wrap-16 per chunk, replicate across the 8 gpsimd cores, group K chunks
    idx_w = loc_sorted_padded[jperm].reshape(G_NCH, G_W16, 16).transpose(0, 2, 1)
    idx_r = np.broadcast_to(
        idx_w.reshape(G_NG, G_K, 1, 16, G_W16), (G_NG, G_K, 8, 16, G_W16)
    )
    return idx_r.transpose(0, 2, 3, 1, 4).reshape(G_NG, P, G_K * G_W16).copy()


def kernel(item_inputs, item_embedding, genre_table):
    idx = np.asarray(item_inputs).astype(np.int64)
    emb = np.asarray(item_embedding, dtype=np.float32)
    gen = np.asarray(genre_table, dtype=np.float32)
    B, V = idx.shape[0], emb.shape[0]

    shard = idx // VSH
    local = idx - shard * VSH
    order = np.argsort(shard * (VSH + 1) + local, kind="stable")
    counts = np.bincount(shard, minlength=8)
    loc_sorted = local[order].astype(np.int16)

    tab = np.zeros((8 * VSH, E), BF16)
    tab[:V, :D] = emb.astype(BF16)
    tab[:V, D:DF] = gen.astype(BF16)
    iota_in = np.arange(P, dtype=np.float32).reshape(P, 1)
    ones_in = np.ones((1, P), BF16)

    # per-core sorted+padded locals and original positions
    locs, positions = [], []
    off = 0
    for c in range(8):
        n = int(counts[c])
        assert n <= CAPC, f"shard {c} overflow: {n} > {CAPC}"
        positions.append(order[off:off + n])
        loc_pad = np.empty(CAPC, np.int16)
        loc_pad[:n] = loc_sorted[off:off + n]
        loc_pad[n:] = loc_sorted[off + n - 1] if n else 0
        locs.append(loc_pad)
        off += n

    try:
        preps = [_host_prep_v3(lp) for lp in locs]
        use_v3 = True
    except AssertionError:
        use_v3 = False

    if use_v3:
        if "nc3" not in _nc_cache:
            _nc_cache["nc3"] = _build_nc_v3()
        nc = _nc_cache["nc3"]
        in_maps = [
            {"idx": ig, "sm": sm, "tab": tab[c * VSH:(c + 1) * VSH],
             "iota": iota_in, "ones": ones_in}
            for c, (ig, sm) in enumerate(preps)
        ]
        jperm = _devperm(R2G, NTG)
        _nc_cache["in_maps"] = in_maps
        _nc_cache["nc"] = nc
        res = run_bass_kernel_spmd(nc, in_maps, core_ids=list(range(8)))
        # single fused cast+scatter: permute the cheap positions array by
        # jperm instead of gathering the 292B rows; padding rows land on a
        # scratch row B that is sliced off.
        out = np.empty((B + 1, DF), np.float32)
        for c in range(8):
            n = len(positions[c])
            posd = np.full(CAPC, B, np.int64)
            posd[jperm[:n]] = positions[c]
            rows = res.results[c]["out"].reshape(CAPC, DF)
            out[posd] = rows
        return out[:B]

    # ---- fallback: full gather ----
    if "nc2" not in _nc_cache:
        _nc_cache["nc2"] = _build_nc_v2()
    nc = _nc_cache["nc2"]
    jperm = _devperm(G_R2, G_NR)
    in_maps = [
        {"idx": _host_prep_v2(locs[c], jperm), "tab": tab[c * VSH:(c + 1) * VSH]}
        for c in range(8)
    ]
    _nc_cache["in_maps"] = in_maps
    _nc_cache["nc"] = nc
    res = run_bass_kernel_spmd(nc, in_maps, core_ids=list(range(8)))
    out = np.empty((B, DF), np.float32)
    for c in range(8):
        rows = res.results[c]["out"].reshape(CAPC, DF)
        out[positions[c]] = rows[: len(positions[c])]
    return out
